# revision 8
# baseline (speedup 1.0000x reference)
"""Trainium2 Bass kernel for nn_CAComm_54829552501030 (sparse_attention).

Math: the reference's attention collapses exactly. With
  s  = upsample2x(parent_x @ conv_kernel + conv_bias)
  Q  = leaf * Wq,  K = s * Wk,  V = s * Wv
  alpha = softmax(scores, axis=-1)                # rows sum to 1
  out[n, i] = sum_j alpha[n, i, j] * V[n, i, 0]   # V broadcasts over the
                                                  # *row* index i (TF bcast)
            = V[n, i, 0] * 1 = s[n, i] * Wv[0, 0]
so the output is exactly  upsample2x(parent_x @ (conv_kernel*Wv) + conv_bias*Wv),
independent of leaf_x / Wq / Wk (verified vs the jax reference, rel err ~1e-7).

Device work (pure data parallel over the 65536 parent pixels, 8 cores):
each core gets 8192 pixels packed as (128, 1024) fp16: partitions hold 8
independent pixel-groups x 16 channels; a 128x128 block-diagonal
stationary matmul (8 copies of the 16x16 conv matrix, built on the HOST
and shipped as a 33KB fp16 header in front of the pixel data) computes
all 8 groups at once. fp16 everywhere halves the DMA bytes vs fp32 and
makes each matmul single-pass (the 2e-2 rel-err budget dwarfs fp16's
~1e-3). The 2x2 nearest upsample is pure duplication, applied while
unsharding on the host.

Schedule notes (from NTFF traces):
  - HBM-read descriptors cost ~95-150ns each mostly-fixed, so input DMAs
    want FEW, BIG per-partition runs: two input DMAs (hdr+c0+c1 = 1288B
    rows on sync/Q1, c2+c3 = 1024B rows on scalar/Q10), not four 512B
    ones. Output writes are cheap even at 512B rows.
  - Every dma_start costs its engine ~0.6us issue + ~0.65us before
    packets flow; queue activation is ~1.5us (Q1) / ~2.2us (Q10) from
    first issue; a completed DMA's semaphore is visible ~0.2-0.7us after
    the last packet. The pipeline overlaps these across 4 column-chunks.
  - The first ACT instruction triggers a 1283ns ACT_TABLE_LOAD: a dummy
    scalar.add at block start pre-warms it off the critical path.
  - An engine's dma_start can be SCHEDULED AHEAD of its own earlier
    compute op (observed: y1's store issued before the ACT bias-add that
    produces it) — every output DMA must wait on an explicit semaphore
    incremented by the op that wrote its data, even same-engine.
  - PE warmup matmuls on uninitialized junk keep the HAM clock gate open
    from engine release until the real matmuls (values never read).
"""

import sys

for _p in ("/opt/trn_rl_repo", "/opt/pypackages"):
    if _p not in sys.path:
        sys.path.append(_p)

import numpy as np

import concourse.bass as bass
import concourse.mybir as mybir
from concourse import bass_utils
from concourse.bass_utils import run_bass_kernel_spmd


def _ensure_trace_support():
    """run_bass_kernel_spmd(trace=True) — e.g. under BASS_TRACE=1 — needs
    antenv.axon_hooks, which this image lacks; register the equivalent
    ctypes NTFF hook so tracing works instead of crashing. Also make the
    post-trace artifact upload non-fatal when no bucket is reachable."""
    import types

    try:
        import antenv.axon_hooks  # noqa: F401
    except ImportError:
        hook = None
        try:
            from trn_agent_boot import trn_boot

            hook = trn_boot._ntff_profile_via_ctypes("/opt/axon/libaxon_pjrt.so")
        except Exception:
            pass
        mod = types.ModuleType("antenv.axon_hooks")
        mod.get_axon_ntff_profile_hook = lambda: hook
        sys.modules["antenv.axon_hooks"] = mod

    orig_upload = bass_utils.upload_artifacts
    if not getattr(orig_upload, "_safe", False):

        def _safe_upload(tmpdir):
            try:
                return orig_upload(tmpdir)
            except Exception:
                return tmpdir

        _safe_upload._safe = True
        bass_utils.upload_artifacts = _safe_upload


_ensure_trace_support()

N_CORES = 8
B, PH, PW, C = 4, 128, 128, 16       # parent_x shape
GROUPS = 128 // C                    # 8 channel-groups per partition dim
PIX_PER_CORE = B * PH * PW // N_CORES  # 8192
NFREE = PIX_PER_CORE // GROUPS       # 1024 pixels per group
HDR = 132                            # 128 blockdiag cols + 1 bias col + 3 pad
CHS = [256, 256, 384, 128]          # chunk col widths (sum = NFREE);
COFF = [0, 256, 512, 896]            # small tail chunk shortens the last
                                     # copy->issue->store latency chain
F16 = mybir.dt.float16
F32 = mybir.dt.float32


def build_nc(warmup: int = 12) -> bass.Bass:
    nc = bass.Bass()
    xh_ext = nc.declare_dram_parameter("xh", [128, HDR + NFREE], F16, isOutput=False)
    y_ext = nc.declare_dram_parameter("y", [128, NFREE], F16, isOutput=True)

    from contextlib import ExitStack

    with ExitStack() as ctx:
        ec = ctx.enter_context
        hx_sb = ec(nc.sbuf_tensor("hx_sb", [128, HDR + NFREE], F16))
        y_sb = ec(nc.sbuf_tensor("y_sb", [128, NFREE], F16))
        bias_f32 = ec(nc.sbuf_tensor("bias_f32", [128, 1], F32))
        junk_sb = ec(nc.sbuf_tensor("junk_sb", [128, 256], F16))
        warm_sb = ec(nc.sbuf_tensor("warm_sb", [128, 1], F16))
        ps0 = ec(nc.psum_tensor("ps0", [128, CHS[0]], F32))
        ps1 = ec(nc.psum_tensor("ps1", [128, CHS[1]], F32))
        ps2 = ec(nc.psum_tensor("ps2", [128, CHS[2]], F32))
        ps3 = ec(nc.psum_tensor("ps3", [128, CHS[3]], F32))
        ps_junk = ec(nc.psum_tensor("ps_junk", [128, 256], F32))
        block = ec(nc.Block())
        dL = ec(nc.semaphore("dL"))
        dR = ec(nc.semaphore("dR"))
        m0 = ec(nc.semaphore("m0"))
        m1 = ec(nc.semaphore("m1"))
        m2 = ec(nc.semaphore("m2"))
        m3 = ec(nc.semaphore("m3"))
        a0 = ec(nc.semaphore("a0"))
        a1 = ec(nc.semaphore("a1"))
        a2 = ec(nc.semaphore("a2"))
        a3 = ec(nc.semaphore("a3"))
        osem = ec(nc.semaphore("osem"))

        S_ap = hx_sb[:, 0:128]            # block-diagonal stationary, fp16
        bias16_ap = hx_sb[:, 128:129]     # per-partition bias, fp16
        SPLIT = HDR + COFF[2]             # sync carries hdr+c0+c1, scalar c2+c3

        def xcol(k):                       # chunk k of the pixel data
            return slice(HDR + COFF[k], HDR + COFF[k] + CHS[k])

        def ycol(k):
            return slice(COFF[k], COFF[k] + CHS[k])

        @block.sync
        def _(sync):
            sync.dma_start(out=hx_sb[:, 0:SPLIT], in_=xh_ext[:, 0:SPLIT]).then_inc(
                dL, 16
            )
            sync.wait_ge(a0, 1)
            sync.dma_start(out=y_ext[:, ycol(0)], in_=y_sb[:, ycol(0)]).then_inc(
                osem, 16
            )
            sync.wait_ge(a2, 1)
            sync.dma_start(out=y_ext[:, ycol(2)], in_=y_sb[:, ycol(2)]).then_inc(
                osem, 16
            )

        @block.scalar
        def _(scalar):
            scalar.dma_start(
                out=hx_sb[:, SPLIT : HDR + NFREE], in_=xh_ext[:, SPLIT : HDR + NFREE]
            ).then_inc(dR, 16)
            # dummy ACT op: absorb the one-time ACT_TABLE_LOAD (~1.3us) here,
            # during the input-DMA wait, instead of on chunk 1's critical path
            scalar.add(warm_sb[:], warm_sb[:], 0.0)
            scalar.wait_ge(m1, 1)
            scalar.add(y_sb[:, ycol(1)], ps1[:], bias16_ap).then_inc(a1, 1)
            scalar.wait_ge(a1, 1)
            scalar.dma_start(out=y_ext[:, ycol(1)], in_=y_sb[:, ycol(1)]).then_inc(
                osem, 16
            )
            scalar.wait_ge(a3, 1)
            scalar.dma_start(out=y_ext[:, ycol(3)], in_=y_sb[:, ycol(3)]).then_inc(
                osem, 16
            )

        @block.tensor
        def _(tensor):
            # Warm-up matmuls read junk_sb UNINITIALIZED: the values are
            # irrelevant (ps_junk is never read) and skipping the memset
            # dependency lets the PE busy-window start at engine release,
            # so the HAM clock-gate flips to 8/8 before the real matmuls.
            for _ in range(warmup):
                tensor.matmul(
                    ps_junk[:], junk_sb[:, 0:128], junk_sb[:],
                    start=True, stop=True, skip_group_check=True,
                )
            tensor.wait_ge(dL, 16)
            tensor.matmul(
                ps0[:], S_ap, hx_sb[:, xcol(0)], start=True, stop=True
            ).then_inc(m0, 1)
            tensor.matmul(
                ps1[:], S_ap, hx_sb[:, xcol(1)], start=True, stop=True
            ).then_inc(m1, 1)
            tensor.wait_ge(dR, 16)
            tensor.matmul(
                ps2[:], S_ap, hx_sb[:, xcol(2)], start=True, stop=True
            ).then_inc(m2, 1)
            tensor.matmul(
                ps3[:], S_ap, hx_sb[:, xcol(3)], start=True, stop=True
            ).then_inc(m3, 1)

        @block.vector
        def _(vector):
            vector.wait_ge(dL, 16)
            vector.tensor_copy(bias_f32[:], bias16_ap)   # fp16 -> fp32 once
            vector.wait_ge(m0, 1)
            vector.tensor_scalar_add(y_sb[:, ycol(0)], ps0[:], bias_f32[:]).then_inc(
                a0, 1
            )
            vector.wait_ge(m2, 1)
            vector.tensor_scalar_add(y_sb[:, ycol(2)], ps2[:], bias_f32[:]).then_inc(
                a2, 1
            )
            vector.wait_ge(m3, 1)
            vector.tensor_scalar_add(y_sb[:, ycol(3)], ps3[:], bias_f32[:]).then_inc(
                a3, 1
            )

    return nc


_NC = None


def _get_nc() -> bass.Bass:
    global _NC
    if _NC is None:
        _NC = build_nc()
    return _NC


def _pack_inputs(parent_x, conv_kernel, conv_bias, Wv):
    wv = float(np.asarray(Wv).reshape(-1)[0])
    W = (np.asarray(conv_kernel, np.float32) * wv).astype(np.float16)    # (16,16)
    bias = (np.asarray(conv_bias, np.float32) * wv).astype(np.float16)   # (16,)

    # header: block-diagonal stationary (out = S.T @ rhs with
    # S[16a+c, 16a+f] = W[c, f]) + per-partition bias column + pad
    hdr = np.zeros((128, HDR), np.float16)
    for a in range(GROUPS):
        hdr[C * a : C * (a + 1), C * a : C * (a + 1)] = W
        hdr[C * a : C * (a + 1), 128] = bias
    # x packed per core: row 16a+c = channel c of pixel-group a
    xf = np.ascontiguousarray(parent_x, dtype=np.float32).reshape(
        N_CORES, GROUPS, NFREE, C
    )
    xp = xf.transpose(0, 1, 3, 2).reshape(N_CORES, 128, NFREE).astype(np.float16)
    xh = np.concatenate([np.broadcast_to(hdr, (N_CORES, 128, HDR)), xp], axis=2)
    return np.ascontiguousarray(xh)


def _make_in_maps(inputs):
    xh = _pack_inputs(
        inputs["parent_x"], inputs["conv_kernel"], inputs["conv_bias"], inputs["Wv"]
    )
    return [{"xh": xh[k]} for k in range(N_CORES)]


def _unpack_output(y_shards):
    # y_shards: (8, 128, 1024) fp16 with row 16a+f = channel f of pixel-group a
    y = np.asarray(y_shards).astype(np.float32).reshape(N_CORES, GROUPS, C, NFREE)
    y = y.transpose(0, 1, 3, 2).reshape(B, PH, PW, C)
    out = np.broadcast_to(
        y[:, :, None, :, None, :], (B, PH, 2, PW, 2, C)
    ).reshape(B, 2 * PH, 2 * PW, C)
    return np.ascontiguousarray(out)


def kernel(parent_x, leaf_x, conv_kernel, conv_bias, Wq, Wk, Wv, **_unused):
    in_maps = _make_in_maps(
        {
            "parent_x": parent_x,
            "conv_kernel": conv_kernel,
            "conv_bias": conv_bias,
            "Wv": Wv,
        }
    )
    nc = _get_nc()
    res = run_bass_kernel_spmd(nc, in_maps, list(range(N_CORES))).results
    y = np.stack([res[k]["y"] for k in range(N_CORES)])
    return _unpack_output(y)


if __name__ == "__main__":
    rng = np.random.default_rng(0)
    inputs = {
        "parent_x": rng.standard_normal((B, PH, PW, C)).astype(np.float32),
        "leaf_x": rng.standard_normal((B, 2 * PH, 2 * PW, C)).astype(np.float32),
        "conv_kernel": (rng.standard_normal((C, C)) * 0.1).astype(np.float32),
        "conv_bias": (rng.standard_normal(C) * 0.1).astype(np.float32),
        "Wq": rng.standard_normal((1, C)).astype(np.float32),
        "Wk": rng.standard_normal((1, C)).astype(np.float32),
        "Wv": rng.standard_normal((1, 1)).astype(np.float32),
    }
    out = kernel(**inputs)
    wv = float(inputs["Wv"][0, 0])
    s = inputs["parent_x"] @ (inputs["conv_kernel"] * wv) + inputs["conv_bias"] * wv
    exp = np.repeat(np.repeat(s, 2, axis=1), 2, axis=2)
    rel = np.linalg.norm(out - exp) / np.linalg.norm(exp)
    print("self-check rel err:", rel)


# revision 9
# speedup vs baseline: 1.0433x; 1.0433x over previous
"""Trainium2 Bass kernel for nn_CAComm_54829552501030 (sparse_attention).

Math: the reference's attention collapses exactly. With
  s  = upsample2x(parent_x @ conv_kernel + conv_bias)
  Q  = leaf * Wq,  K = s * Wk,  V = s * Wv
  alpha = softmax(scores, axis=-1)                # rows sum to 1
  out[n, i] = sum_j alpha[n, i, j] * V[n, i, 0]   # V broadcasts over the
                                                  # *row* index i (TF bcast)
            = V[n, i, 0] * 1 = s[n, i] * Wv[0, 0]
so the output is exactly  upsample2x(parent_x @ (conv_kernel*Wv) + conv_bias*Wv),
independent of leaf_x / Wq / Wk (verified vs the jax reference, rel err ~1e-7).

Device work (pure data parallel over the 65536 parent pixels, 8 cores):
each core gets 8192 pixels packed as (128, 1024) fp16: partitions hold 8
independent pixel-groups x 16 channels; a 128x128 block-diagonal
stationary matmul (8 copies of the 16x16 conv matrix, built on the HOST
and shipped as a 33KB fp16 header in front of the pixel data) computes
all 8 groups at once. fp16 everywhere halves the DMA bytes vs fp32 and
makes each matmul single-pass (the 2e-2 rel-err budget dwarfs fp16's
~1e-3). The 2x2 nearest upsample is pure duplication, applied while
unsharding on the host.

Schedule notes (from NTFF traces):
  - HBM-read descriptors cost ~95-150ns each mostly-fixed, so input DMAs
    want FEW, BIG per-partition runs: two input DMAs (hdr+c0+c1 = 1288B
    rows on sync/Q1, c2+c3 = 1024B rows on scalar/Q10), not four 512B
    ones. Output writes are cheap even at 512B rows.
  - Every dma_start costs its engine ~0.6us issue + ~0.65us before
    packets flow; queue activation is ~1.5us (Q1) / ~2.2us (Q10) from
    first issue; a completed DMA's semaphore is visible ~0.2-0.7us after
    the last packet. The pipeline overlaps these across 4 column-chunks.
  - The first ACT instruction triggers a 1283ns ACT_TABLE_LOAD: a dummy
    scalar.add at block start pre-warms it off the critical path.
  - An engine's dma_start can be SCHEDULED AHEAD of its own earlier
    compute op (observed: y1's store issued before the ACT bias-add that
    produces it) — every output DMA must wait on an explicit semaphore
    incremented by the op that wrote its data, even same-engine.
  - PE warmup matmuls on uninitialized junk keep the HAM clock gate open
    from engine release until the real matmuls (values never read).
"""

import sys

for _p in ("/opt/trn_rl_repo", "/opt/pypackages"):
    if _p not in sys.path:
        sys.path.append(_p)

import numpy as np

import concourse.bass as bass
import concourse.mybir as mybir
from concourse import bass_utils
from concourse.bass_utils import run_bass_kernel_spmd


def _ensure_trace_support():
    """run_bass_kernel_spmd(trace=True) — e.g. under BASS_TRACE=1 — needs
    antenv.axon_hooks, which this image lacks; register the equivalent
    ctypes NTFF hook so tracing works instead of crashing. Also make the
    post-trace artifact upload non-fatal when no bucket is reachable."""
    import types

    try:
        import antenv.axon_hooks  # noqa: F401
    except ImportError:
        hook = None
        try:
            from trn_agent_boot import trn_boot

            hook = trn_boot._ntff_profile_via_ctypes("/opt/axon/libaxon_pjrt.so")
        except Exception:
            pass
        mod = types.ModuleType("antenv.axon_hooks")
        mod.get_axon_ntff_profile_hook = lambda: hook
        sys.modules["antenv.axon_hooks"] = mod

    orig_upload = bass_utils.upload_artifacts
    if not getattr(orig_upload, "_safe", False):

        def _safe_upload(tmpdir):
            try:
                return orig_upload(tmpdir)
            except Exception:
                return tmpdir

        _safe_upload._safe = True
        bass_utils.upload_artifacts = _safe_upload


_ensure_trace_support()

N_CORES = 8
B, PH, PW, C = 4, 128, 128, 16       # parent_x shape
GROUPS = 128 // C                    # 8 channel-groups per partition dim
PIX_PER_CORE = B * PH * PW // N_CORES  # 8192
NFREE = PIX_PER_CORE // GROUPS       # 1024 pixels per group
HDR = 132                            # 128 blockdiag cols + 1 bias col + 3 pad
CHS = [256, 256, 384, 128]          # chunk col widths (sum = NFREE);
COFF = [0, 256, 512, 896]            # small tail chunk shortens the last
                                     # copy->issue->store latency chain
F16 = mybir.dt.float16
F32 = mybir.dt.float32


def build_nc(warmup: int = 12) -> bass.Bass:
    nc = bass.Bass()
    # separate params so each input DMA reads a fully CONTIGUOUS DRAM block
    # (row stride == run length -> better HBM locality than column slices)
    xl_ext = nc.declare_dram_parameter("xl", [128, HDR + COFF[2]], F16, isOutput=False)
    xr_ext = nc.declare_dram_parameter("xr", [128, NFREE - COFF[2]], F16, isOutput=False)
    y_ext = nc.declare_dram_parameter("y", [128, NFREE], F16, isOutput=True)

    from contextlib import ExitStack

    with ExitStack() as ctx:
        ec = ctx.enter_context
        hx_sb = ec(nc.sbuf_tensor("hx_sb", [128, HDR + NFREE], F16))
        y_sb = ec(nc.sbuf_tensor("y_sb", [128, NFREE], F16))
        bias_f32 = ec(nc.sbuf_tensor("bias_f32", [128, 1], F32))
        junk_sb = ec(nc.sbuf_tensor("junk_sb", [128, 256], F16))
        warm_sb = ec(nc.sbuf_tensor("warm_sb", [128, 1], F16))
        ps0 = ec(nc.psum_tensor("ps0", [128, CHS[0]], F32))
        ps1 = ec(nc.psum_tensor("ps1", [128, CHS[1]], F32))
        ps2 = ec(nc.psum_tensor("ps2", [128, CHS[2]], F32))
        ps3 = ec(nc.psum_tensor("ps3", [128, CHS[3]], F32))
        ps_junk = ec(nc.psum_tensor("ps_junk", [128, 256], F32))
        block = ec(nc.Block())
        dL = ec(nc.semaphore("dL"))
        dR = ec(nc.semaphore("dR"))
        m0 = ec(nc.semaphore("m0"))
        m1 = ec(nc.semaphore("m1"))
        m2 = ec(nc.semaphore("m2"))
        m3 = ec(nc.semaphore("m3"))
        a0 = ec(nc.semaphore("a0"))
        a1 = ec(nc.semaphore("a1"))
        a2 = ec(nc.semaphore("a2"))
        a3 = ec(nc.semaphore("a3"))
        osem = ec(nc.semaphore("osem"))

        S_ap = hx_sb[:, 0:128]            # block-diagonal stationary, fp16
        bias16_ap = hx_sb[:, 128:129]     # per-partition bias, fp16
        SPLIT = HDR + COFF[2]             # sync carries hdr+c0+c1, scalar c2+c3

        def xcol(k):                       # chunk k of the pixel data
            return slice(HDR + COFF[k], HDR + COFF[k] + CHS[k])

        def ycol(k):
            return slice(COFF[k], COFF[k] + CHS[k])

        @block.sync
        def _(sync):
            sync.dma_start(out=hx_sb[:, 0:SPLIT], in_=xl_ext[:]).then_inc(dL, 16)
            sync.wait_ge(a0, 1)
            sync.dma_start(out=y_ext[:, ycol(0)], in_=y_sb[:, ycol(0)]).then_inc(
                osem, 16
            )
            sync.wait_ge(a2, 1)
            sync.dma_start(out=y_ext[:, ycol(2)], in_=y_sb[:, ycol(2)]).then_inc(
                osem, 16
            )

        @block.scalar
        def _(scalar):
            scalar.dma_start(
                out=hx_sb[:, SPLIT : HDR + NFREE], in_=xr_ext[:]
            ).then_inc(dR, 16)
            # dummy ACT op: absorb the one-time ACT_TABLE_LOAD (~1.3us) here,
            # during the input-DMA wait, instead of on chunk 1's critical path
            scalar.add(warm_sb[:], warm_sb[:], 0.0)
            scalar.wait_ge(m1, 1)
            scalar.add(y_sb[:, ycol(1)], ps1[:], bias16_ap).then_inc(a1, 1)
            scalar.wait_ge(a1, 1)
            scalar.dma_start(out=y_ext[:, ycol(1)], in_=y_sb[:, ycol(1)]).then_inc(
                osem, 16
            )
            scalar.wait_ge(a3, 1)
            scalar.dma_start(out=y_ext[:, ycol(3)], in_=y_sb[:, ycol(3)]).then_inc(
                osem, 16
            )

        @block.tensor
        def _(tensor):
            # Warm-up matmuls read junk_sb UNINITIALIZED: the values are
            # irrelevant (ps_junk is never read) and skipping the memset
            # dependency lets the PE busy-window start at engine release,
            # so the HAM clock-gate flips to 8/8 before the real matmuls.
            for _ in range(warmup):
                tensor.matmul(
                    ps_junk[:], junk_sb[:, 0:128], junk_sb[:],
                    start=True, stop=True, skip_group_check=True,
                )
            tensor.wait_ge(dL, 16)
            tensor.matmul(
                ps0[:], S_ap, hx_sb[:, xcol(0)], start=True, stop=True
            ).then_inc(m0, 1)
            tensor.matmul(
                ps1[:], S_ap, hx_sb[:, xcol(1)], start=True, stop=True
            ).then_inc(m1, 1)
            tensor.wait_ge(dR, 16)
            tensor.matmul(
                ps2[:], S_ap, hx_sb[:, xcol(2)], start=True, stop=True
            ).then_inc(m2, 1)
            tensor.matmul(
                ps3[:], S_ap, hx_sb[:, xcol(3)], start=True, stop=True
            ).then_inc(m3, 1)

        @block.vector
        def _(vector):
            vector.wait_ge(dL, 16)
            vector.tensor_copy(bias_f32[:], bias16_ap)   # fp16 -> fp32 once
            vector.wait_ge(m0, 1)
            vector.tensor_scalar_add(y_sb[:, ycol(0)], ps0[:], bias_f32[:]).then_inc(
                a0, 1
            )
            vector.wait_ge(m2, 1)
            vector.tensor_scalar_add(y_sb[:, ycol(2)], ps2[:], bias_f32[:]).then_inc(
                a2, 1
            )
            vector.wait_ge(m3, 1)
            vector.tensor_scalar_add(y_sb[:, ycol(3)], ps3[:], bias_f32[:]).then_inc(
                a3, 1
            )

    return nc


_NC = None


def _get_nc() -> bass.Bass:
    global _NC
    if _NC is None:
        _NC = build_nc()
    return _NC


def _pack_inputs(parent_x, conv_kernel, conv_bias, Wv):
    wv = float(np.asarray(Wv).reshape(-1)[0])
    W = (np.asarray(conv_kernel, np.float32) * wv).astype(np.float16)    # (16,16)
    bias = (np.asarray(conv_bias, np.float32) * wv).astype(np.float16)   # (16,)

    # header: block-diagonal stationary (out = S.T @ rhs with
    # S[16a+c, 16a+f] = W[c, f]) + per-partition bias column + pad
    hdr = np.zeros((128, HDR), np.float16)
    for a in range(GROUPS):
        hdr[C * a : C * (a + 1), C * a : C * (a + 1)] = W
        hdr[C * a : C * (a + 1), 128] = bias
    # x packed per core: row 16a+c = channel c of pixel-group a
    xf = np.ascontiguousarray(parent_x, dtype=np.float32).reshape(
        N_CORES, GROUPS, NFREE, C
    )
    xp = xf.transpose(0, 1, 3, 2).reshape(N_CORES, 128, NFREE).astype(np.float16)
    xh = np.concatenate([np.broadcast_to(hdr, (N_CORES, 128, HDR)), xp], axis=2)
    split = HDR + COFF[2]
    return np.ascontiguousarray(xh[:, :, :split]), np.ascontiguousarray(xh[:, :, split:])


def _make_in_maps(inputs):
    xl, xr = _pack_inputs(
        inputs["parent_x"], inputs["conv_kernel"], inputs["conv_bias"], inputs["Wv"]
    )
    return [{"xl": xl[k], "xr": xr[k]} for k in range(N_CORES)]


def _unpack_output(y_shards):
    # y_shards: (8, 128, 1024) fp16 with row 16a+f = channel f of pixel-group a
    y = np.asarray(y_shards).astype(np.float32).reshape(N_CORES, GROUPS, C, NFREE)
    y = y.transpose(0, 1, 3, 2).reshape(B, PH, PW, C)
    out = np.broadcast_to(
        y[:, :, None, :, None, :], (B, PH, 2, PW, 2, C)
    ).reshape(B, 2 * PH, 2 * PW, C)
    return np.ascontiguousarray(out)


def kernel(parent_x, leaf_x, conv_kernel, conv_bias, Wq, Wk, Wv, **_unused):
    in_maps = _make_in_maps(
        {
            "parent_x": parent_x,
            "conv_kernel": conv_kernel,
            "conv_bias": conv_bias,
            "Wv": Wv,
        }
    )
    nc = _get_nc()
    res = run_bass_kernel_spmd(nc, in_maps, list(range(N_CORES))).results
    y = np.stack([res[k]["y"] for k in range(N_CORES)])
    return _unpack_output(y)


if __name__ == "__main__":
    rng = np.random.default_rng(0)
    inputs = {
        "parent_x": rng.standard_normal((B, PH, PW, C)).astype(np.float32),
        "leaf_x": rng.standard_normal((B, 2 * PH, 2 * PW, C)).astype(np.float32),
        "conv_kernel": (rng.standard_normal((C, C)) * 0.1).astype(np.float32),
        "conv_bias": (rng.standard_normal(C) * 0.1).astype(np.float32),
        "Wq": rng.standard_normal((1, C)).astype(np.float32),
        "Wk": rng.standard_normal((1, C)).astype(np.float32),
        "Wv": rng.standard_normal((1, 1)).astype(np.float32),
    }
    out = kernel(**inputs)
    wv = float(inputs["Wv"][0, 0])
    s = inputs["parent_x"] @ (inputs["conv_kernel"] * wv) + inputs["conv_bias"] * wv
    exp = np.repeat(np.repeat(s, 2, axis=1), 2, axis=2)
    rel = np.linalg.norm(out - exp) / np.linalg.norm(exp)
    print("self-check rel err:", rel)
